# revision 12
# baseline (speedup 1.0000x reference)
"""Attention pooling kernel for Trainium2 (8 NeuronCores) — fp16 pipeline.

Computes: scores = E @ q; w = softmax(scores); out = w @ E
for E [N=2097152, 64] fp32, q [64] fp32.

Strategy (per core, N/8 = 262144 rows), all heavy traffic in fp16:
  - Host casts the core's row-shard to fp16 and packs it as
    ep [128, 131072]: with n = 16*f + j (j in [0,16)) and d = 8*g + e
    (e in [0,8)), partition k = 8*j + e holds E[16f+j, 8g+e] at column
    ch*8192 + g*1024 + fl  (f = ch*1024 + fl; 16 chunks of 8192 cols).
  - Scores: 8 chained matmuls per 512-col half (stationary Qg [128,128],
    Qg[8j+e, 8j'+e'] = q[8g+e]*(j==j')) accumulate in PSUM ->
    scores s(16f+j) land replicated over the 8 e'-slots of parity j.
    exp therefore only touches [128, F] per chunk — 8x less ACT work
    than a 2-parity layout.
  - ACT: w = exp(scores - C) -> fp16, fused accum_out gives sumexp
    partials (C computed from q alone; cancels in the final division).
  - DVE: per chunk 8 tensor_tensor fp16 multiplies (2x perf mode):
    prod_g = ep_g * w.  The f-reduction of prod_g is split across three
    otherwise-idle resources so no engine exceeds the DMA pace:
      g=0..3 -> ACT Copy with accum_out over 2-chunk spans
      g=4..7 -> PE identity-stationary matmuls (fp8 identity: cheap
                LDWEIGHTS) accumulating into 4 persistent PSUM banks,
                emitted one chunk late so PE never stalls on the
                exp->mult chain.
    The last span's ACT groups instead use fused DVE
    scalar_tensor_tensor (mult+accum) to shorten the drain tail.
  - Host: out[d] = sum_cores sum_j acc[(j,e),g] / sum exp, d = 8g+e.
"""

import sys

sys.path.insert(0, "/opt/trn_rl_repo")

import numpy as np

N_TOTAL = 2097152
D = 64
N_CORES = 8
N_PER_CORE = N_TOTAL // N_CORES          # 262144
J = 16                                   # parities (n mod 16)
G = 8                                    # dim groups of 8
F_TOT = N_PER_CORE // J                  # 16384 f-columns per core
F = 1024                                 # f-columns per chunk
N_CHUNKS = F_TOT // F                    # 16
CH_COLS = G * F                          # 8192 packed cols per chunk
COLS = F_TOT * G                         # 131072 packed cols per core
SPAN = 2                                 # chunks per ACT/GPS reduce span
G_ACT = 4                                # groups 0..3 reduced on ACT
G_GPS = 0                                # (gpsimd pool rejected by walrus)
MM_N = 512                               # matmul free dim (one PSUM bank)
RES_COLS = 9

_compiled = {}


def _build_nc():
    import concourse.bacc as bacc
    import concourse.bass as bass
    import concourse.mybir as mybir
    import concourse.tile as tile

    fp32 = mybir.dt.float32
    f16 = mybir.dt.float16
    f8 = mybir.dt.float8e4

    nc = bacc.Bacc()
    ep_dram = nc.declare_dram_parameter("epack", [128, COLS], f16, isOutput=False)
    qm_dram = nc.declare_dram_parameter("qmats", [128, 128 * G], f16, isOutput=False)
    id_dram = nc.declare_dram_parameter("ident", [128, 128], f8, isOutput=False)
    cs_dram = nc.declare_dram_parameter("cshift", [128, 1], fp32, isOutput=False)
    out_dram = nc.declare_dram_parameter("out", [128, RES_COLS], fp32, isOutput=True)

    n_halves = F // MM_N
    G_PE = G - G_ACT
    PE0 = G_ACT                           # first PE group index
    LAST_SPAN_T0 = N_CHUNKS - SPAN        # chunks >= this use the drain path

    with tile.TileContext(nc) as tc:
        with (
            tc.tile_pool(name="const", bufs=1) as const_pool,
            tc.tile_pool(name="ep", bufs=5) as ep_pool,
            tc.tile_pool(name="w", bufs=3) as w_pool,
            tc.tile_pool(name="prodA", bufs=3) as prodA_pool,
            tc.tile_pool(name="prodS", bufs=2) as prodS_pool,
            tc.tile_pool(name="junk", bufs=1) as junk_pool,
            tc.tile_pool(name="sm", bufs=3) as sm_pool,
            tc.tile_pool(name="racc", bufs=2) as racc_pool,
            tc.tile_pool(name="master", bufs=1) as master_pool,
            tc.tile_pool(name="ps", bufs=2, space=bass.MemorySpace.PSUM) as ps_pool,
            tc.tile_pool(name="acc", bufs=1, space=bass.MemorySpace.PSUM) as acc_pool,
        ):
            qmats = const_pool.tile([128, 128 * G], f16, tag="qmats")
            ident = const_pool.tile([128, 128], f8, tag="ident")
            cshift = const_pool.tile([128, 1], fp32, tag="cshift")
            nc.sync.dma_start(cshift[:], cs_dram[:])
            nc.sync.dma_start(qmats[:], qm_dram[:])
            nc.sync.dma_start(ident[:], id_dram[:])

            # touch exp early so the ACT table load runs during the first
            # ep DMA instead of on the first real exp
            warm = const_pool.tile([128, 1], fp32, tag="warm")
            nc.scalar.activation(
                warm[:], cshift[:], mybir.ActivationFunctionType.Exp,
                bias=cshift[:, 0:1], scale=1.0,
            )

            master4 = master_pool.tile([128, G_ACT], fp32, tag="master4")
            master_se = master_pool.tile([128, 1], fp32, tag="master_se")
            stt_acc = master_pool.tile([128, SPAN * G_ACT], fp32, tag="stt_acc")
            accP = [
                acc_pool.tile([128, MM_N], fp32, tag=f"accP{gi}", name=f"accP{gi}")
                for gi in range(G_PE)
            ]

            pend = None          # (t, [prodS tiles]) awaiting PE reduce
            pend_act = None      # (prodA tile, racc tile) awaiting ACT reduce
            prodA = None

            def emit_pe_reduce(prev_t, tiles):
                for gi, pt in enumerate(tiles):
                    for h in range(n_halves):
                        nc.tensor.matmul(
                            accP[gi][:],
                            ident[:],
                            pt[:, h * MM_N:(h + 1) * MM_N],
                            start=(prev_t == 0 and h == 0),
                            stop=(prev_t == N_CHUNKS - 1 and h == n_halves - 1),
                        )

            for t in range(N_CHUNKS):
                ep = ep_pool.tile([128, CH_COLS], f16, tag="ep")
                dma_eng = nc.sync if t % 2 == 0 else nc.scalar
                if t == 0:
                    # per-slab DMAs so the first score matmuls start early
                    for g in range(G):
                        nc.sync.dma_start(
                            ep[:, g * F:(g + 1) * F],
                            ep_dram[:, g * F:(g + 1) * F],
                        )
                else:
                    dma_eng.dma_start(
                        ep[:], ep_dram[:, t * CH_COLS:(t + 1) * CH_COLS]
                    )

                ps = ps_pool.tile([128, F], fp32, tag="ps")
                for h in range(n_halves):
                    lo = h * MM_N
                    for g in range(G):
                        nc.tensor.matmul(
                            ps[:, lo:lo + MM_N],
                            qmats[:, g * 128:(g + 1) * 128],
                            ep[:, g * F + lo:g * F + lo + MM_N],
                            start=(g == 0),
                            stop=(g == G - 1),
                        )
                # PE reduce of the previous chunk's prod tiles (one chunk
                # late so PE doesn't stall on the exp->mult chain)
                if pend is not None:
                    emit_pe_reduce(*pend)

                w = w_pool.tile([128, F], f16, tag="w")
                se = sm_pool.tile([128, 1], fp32, tag="se")
                nc.scalar.activation(
                    w[:],
                    ps[:],
                    mybir.ActivationFunctionType.Exp,
                    bias=cshift[:, 0:1],
                    scale=1.0,
                    accum_out=se[:],
                )
                # two ACT span-reduce ops per chunk, for the previous span
                if pend_act is not None:
                    pa, racc = pend_act
                    for g in (0, 1) if t % SPAN == 0 else (2, 3):
                        junk = junk_pool.tile([128, SPAN * F], f16, tag="junk")
                        nc.scalar.activation(
                            junk[:],
                            pa[:, g * SPAN * F:(g + 1) * SPAN * F],
                            mybir.ActivationFunctionType.Copy,
                            bias=0.0,
                            scale=1.0,
                            accum_out=racc[:, g:g + 1],
                        )
                    if t % SPAN == SPAN - 1:
                        if t == SPAN + SPAN - 1:
                            nc.vector.tensor_copy(master4[:], racc[:])
                        else:
                            nc.vector.tensor_add(master4[:], master4[:], racc[:])
                        pend_act = None
                if t == 0:
                    nc.vector.tensor_copy(master_se[:], se[:])
                else:
                    nc.vector.tensor_add(master_se[:], master_se[:], se[:])

                # PE-path mults first: the next chunk's PE reduce waits on them
                cur = []
                for g in range(PE0, G):
                    pS = prodS_pool.tile([128, F], f16, tag=f"prodS{g}")
                    nc.vector.tensor_mul(pS[:], ep[:, g * F:(g + 1) * F], w[:])
                    cur.append(pS)
                pend = (t, cur)

                # ACT-path mults (fused stt accum in the last span)
                if t < LAST_SPAN_T0:
                    if t % SPAN == 0:
                        prodA = prodA_pool.tile(
                            [128, G_ACT * SPAN * F], f16, tag="prodA"
                        )
                    for g in range(G_ACT):
                        dst = prodA[
                            :,
                            (g * SPAN + (t % SPAN)) * F:(g * SPAN + (t % SPAN) + 1) * F,
                        ]
                        nc.vector.tensor_mul(dst, ep[:, g * F:(g + 1) * F], w[:])
                    if t % SPAN == SPAN - 1:
                        racc = racc_pool.tile([128, G_ACT], fp32, tag="racc")
                        pend_act = (prodA, racc)
                else:
                    for g in range(G_ACT):
                        junk = junk_pool.tile([128, F], f16, tag="sjunk")
                        nc.vector.scalar_tensor_tensor(
                            junk[:],
                            ep[:, g * F:(g + 1) * F],
                            1.0,
                            w[:],
                            op0=mybir.AluOpType.mult,
                            op1=mybir.AluOpType.mult,
                            accum_out=stt_acc[:, (t - LAST_SPAN_T0) * G_ACT + g:
                                              (t - LAST_SPAN_T0) * G_ACT + g + 1],
                        )

            # drain: last chunk's PE reduce, stt-tail accums, final combine
            emit_pe_reduce(*pend)
            assert pend_act is None
            for sl in range(SPAN):
                nc.vector.tensor_add(
                    master4[:], master4[:],
                    stt_acc[:, sl * G_ACT:(sl + 1) * G_ACT],
                )

            res = master_pool.tile([128, RES_COLS], fp32, tag="res")
            nc.vector.tensor_copy(res[:, 0:G_ACT], master4[:])
            for gi in range(G_PE):
                nc.vector.tensor_reduce(
                    res[:, G_ACT + gi:G_ACT + gi + 1],
                    accP[gi][:],
                    axis=mybir.AxisListType.X,
                    op=mybir.AluOpType.add,
                )
            nc.vector.tensor_copy(res[:, 8:9], master_se[:])
            nc.sync.dma_start(out_dram[:], res[:])

    nc.compile()
    return nc


def _pack_core(e_core):
    """[N_PER_CORE, 64] fp32 -> [128, COLS] fp16 in the (j, e, g) layout."""
    a = e_core.reshape(N_CHUNKS, F, J, G, 8)        # [ch, fl, j, g, e]
    a = a.transpose(0, 2, 4, 3, 1)                  # [ch, j, e, g, fl]
    a = a.reshape(N_CHUNKS, 128, CH_COLS)
    a = a.transpose(1, 0, 2).reshape(128, COLS)
    return np.ascontiguousarray(a).astype(np.float16)


def _make_consts(query):
    import ml_dtypes

    c_shift = float(6.0 * np.linalg.norm(query))
    qmats = np.zeros((128, 128 * G), dtype=np.float32)
    for g in range(G):
        for j in range(J):
            qmats[8 * j:8 * j + 8, g * 128 + 8 * j:g * 128 + 8 * j + 8] = (
                query[8 * g:8 * g + 8][:, None]
            )
    ident = np.eye(128, dtype=np.float32)
    cshift = np.full((128, 1), -c_shift, dtype=np.float32)
    return (
        qmats.astype(np.float16),
        ident.astype(ml_dtypes.float8_e4m3),
        cshift,
    )


def build_in_maps(embeddings, query):
    embeddings = np.asarray(embeddings, dtype=np.float32)
    query = np.asarray(query, dtype=np.float32)
    qmats, ident, cshift = _make_consts(query)
    in_maps = []
    for c in range(N_CORES):
        e_core = embeddings[c * N_PER_CORE:(c + 1) * N_PER_CORE]
        in_maps.append({
            "epack": _pack_core(e_core),
            "qmats": qmats,
            "ident": ident,
            "cshift": cshift,
        })
    return in_maps


def combine_results(results):
    num = np.zeros(D, dtype=np.float64)
    z = 0.0
    for r in results:
        o = r["out"].astype(np.float64)              # [128, 9]
        for g in range(G):
            num[8 * g:8 * g + 8] += o[:, g].reshape(J, 8).sum(axis=0)
        z += o[0:128:8, 8].sum()
    return (num / z).astype(np.float32)


def get_nc():
    if "nc" not in _compiled:
        _compiled["nc"] = _build_nc()
    return _compiled["nc"]


def kernel(embeddings, query):
    from concourse.bass_utils import run_bass_kernel_spmd

    nc = get_nc()
    in_maps = build_in_maps(embeddings, query)

    res = None
    for attempt in range(3):
        try:
            res = run_bass_kernel_spmd(nc, in_maps, list(range(N_CORES)))
            break
        except Exception:
            if attempt == 2:
                raise

    return combine_results(res.results)


# revision 13
# speedup vs baseline: 1.0319x; 1.0319x over previous
"""Attention pooling kernel for Trainium2 (8 NeuronCores) — fp16 pipeline.

Computes: scores = E @ q; w = softmax(scores); out = w @ E
for E [N=2097152, 64] fp32, q [64] fp32.

Strategy (per core, N/8 = 262144 rows), all heavy traffic in fp16:
  - Host casts the core's row-shard to fp16 and packs it as
    ep [128, 131072]: with n = 16*f + j (j in [0,16)) and d = 8*g + e
    (e in [0,8)), partition k = 8*j + e holds E[16f+j, 8g+e] at column
    ch*8192 + g*1024 + fl  (f = ch*1024 + fl; 16 chunks of 8192 cols).
  - Scores: 8 chained matmuls per 512-col half (stationary Qg [128,128],
    Qg[8j+e, 8j'+e'] = q[8g+e]*(j==j')) accumulate in PSUM ->
    scores s(16f+j) land replicated over the 8 e'-slots of parity j.
    exp therefore only touches [128, F] per chunk — 8x less ACT work
    than a 2-parity layout.
  - ACT: w = exp(scores - C) -> fp16, fused accum_out gives sumexp
    partials (C computed from q alone; cancels in the final division).
  - DVE: per chunk 8 tensor_tensor fp16 multiplies (2x perf mode):
    prod_g = ep_g * w.  The f-reduction of prod_g is split across three
    otherwise-idle resources so no engine exceeds the DMA pace:
      g=0..3 -> ACT Copy with accum_out over 2-chunk spans
      g=4..7 -> PE identity-stationary matmuls (fp8 identity: cheap
                LDWEIGHTS) accumulating into 4 persistent PSUM banks,
                emitted one chunk late so PE never stalls on the
                exp->mult chain.
    The last span's ACT groups instead use fused DVE
    scalar_tensor_tensor (mult+accum) to shorten the drain tail.
  - Host: out[d] = sum_cores sum_j acc[(j,e),g] / sum exp, d = 8g+e.
"""

import sys

sys.path.insert(0, "/opt/trn_rl_repo")

import numpy as np

N_TOTAL = 2097152
D = 64
N_CORES = 8
N_PER_CORE = N_TOTAL // N_CORES          # 262144
J = 16                                   # parities (n mod 16)
G = 8                                    # dim groups of 8
F_TOT = N_PER_CORE // J                  # 16384 f-columns per core
F = 1024                                 # f-columns per chunk
N_CHUNKS = F_TOT // F                    # 16
CH_COLS = G * F                          # 8192 packed cols per chunk
COLS = F_TOT * G                         # 131072 packed cols per core
SPAN = 2                                 # chunks per ACT/GPS reduce span
G_ACT = 4                                # groups 0..3 reduced on ACT
G_GPS = 0                                # (gpsimd pool rejected by walrus)
MM_N = 512                               # matmul free dim (one PSUM bank)
RES_COLS = 9

_compiled = {}


def _build_nc():
    import concourse.bacc as bacc
    import concourse.bass as bass
    import concourse.mybir as mybir
    import concourse.tile as tile

    fp32 = mybir.dt.float32
    f16 = mybir.dt.float16
    f8 = mybir.dt.float8e4

    nc = bacc.Bacc()
    ep_dram = nc.declare_dram_parameter("epack", [128, COLS], f16, isOutput=False)
    qm_dram = nc.declare_dram_parameter("qmats", [128, 128 * G], f16, isOutput=False)
    id_dram = nc.declare_dram_parameter("ident", [128, 128], f8, isOutput=False)
    cs_dram = nc.declare_dram_parameter("cshift", [128, 1], fp32, isOutput=False)
    out_dram = nc.declare_dram_parameter("out", [128, RES_COLS], fp32, isOutput=True)

    n_halves = F // MM_N
    G_PE = G - G_ACT
    PE0 = G_ACT                           # first PE group index
    LAST_SPAN_T0 = N_CHUNKS - SPAN        # chunks >= this use the drain path

    with tile.TileContext(nc) as tc:
        with (
            tc.tile_pool(name="const", bufs=1) as const_pool,
            tc.tile_pool(name="ep", bufs=5) as ep_pool,
            tc.tile_pool(name="w", bufs=3) as w_pool,
            tc.tile_pool(name="prodA", bufs=3) as prodA_pool,
            tc.tile_pool(name="prodS", bufs=2) as prodS_pool,
            tc.tile_pool(name="junk", bufs=1) as junk_pool,
            tc.tile_pool(name="sm", bufs=3) as sm_pool,
            tc.tile_pool(name="racc", bufs=2) as racc_pool,
            tc.tile_pool(name="master", bufs=1) as master_pool,
            tc.tile_pool(name="ps", bufs=2, space=bass.MemorySpace.PSUM) as ps_pool,
            tc.tile_pool(name="acc", bufs=1, space=bass.MemorySpace.PSUM) as acc_pool,
        ):
            qmats = const_pool.tile([128, 128 * G], f16, tag="qmats")
            ident = const_pool.tile([128, 128], f8, tag="ident")
            cshift = const_pool.tile([128, 1], fp32, tag="cshift")
            nc.sync.dma_start(cshift[:], cs_dram[:])
            nc.sync.dma_start(qmats[:], qm_dram[:])
            nc.sync.dma_start(ident[:], id_dram[:])

            # touch exp early so the ACT table load runs during the first
            # ep DMA instead of on the first real exp
            warm = const_pool.tile([128, 1], fp32, tag="warm")
            nc.scalar.activation(
                warm[:], cshift[:], mybir.ActivationFunctionType.Exp,
                bias=cshift[:, 0:1], scale=1.0,
            )

            master4 = master_pool.tile([128, G_ACT], fp32, tag="master4")
            master_se = master_pool.tile([128, 1], fp32, tag="master_se")
            stt_acc = master_pool.tile([128, SPAN * G_ACT], fp32, tag="stt_acc")
            accP = [
                acc_pool.tile([128, MM_N], fp32, tag=f"accP{gi}", name=f"accP{gi}")
                for gi in range(G_PE)
            ]

            pend = None          # (t, [prodS tiles]) awaiting PE reduce
            pend_act = None      # (prodA tile, racc tile) awaiting ACT reduce
            prodA = None

            def emit_pe_reduce(prev_t, tiles):
                for gi, pt in enumerate(tiles):
                    for h in range(n_halves):
                        nc.tensor.matmul(
                            accP[gi][:],
                            ident[:],
                            pt[:, h * MM_N:(h + 1) * MM_N],
                            start=(prev_t == 0 and h == 0),
                            stop=(prev_t == N_CHUNKS - 1 and h == n_halves - 1),
                        )

            for t in range(N_CHUNKS):
                ep = ep_pool.tile([128, CH_COLS], f16, tag="ep")
                if t == 0:
                    # per-slab DMAs so the first score matmuls start early
                    for g in range(G):
                        nc.sync.dma_start(
                            ep[:, g * F:(g + 1) * F],
                            ep_dram[:, g * F:(g + 1) * F],
                        )
                else:
                    nc.sync.dma_start(
                        ep[:], ep_dram[:, t * CH_COLS:(t + 1) * CH_COLS]
                    )

                ps = ps_pool.tile([128, F], fp32, tag="ps")
                for h in range(n_halves):
                    lo = h * MM_N
                    for g in range(G):
                        nc.tensor.matmul(
                            ps[:, lo:lo + MM_N],
                            qmats[:, g * 128:(g + 1) * 128],
                            ep[:, g * F + lo:g * F + lo + MM_N],
                            start=(g == 0),
                            stop=(g == G - 1),
                        )
                # PE reduce of the previous chunk's prod tiles (one chunk
                # late so PE doesn't stall on the exp->mult chain)
                if pend is not None:
                    emit_pe_reduce(*pend)

                w = w_pool.tile([128, F], f16, tag="w")
                se = sm_pool.tile([128, 1], fp32, tag="se")
                nc.scalar.activation(
                    w[:],
                    ps[:],
                    mybir.ActivationFunctionType.Exp,
                    bias=cshift[:, 0:1],
                    scale=1.0,
                    accum_out=se[:],
                )
                # two ACT span-reduce ops per chunk, for the previous span
                if pend_act is not None:
                    pa, racc = pend_act
                    for g in (0, 1) if t % SPAN == 0 else (2, 3):
                        junk = junk_pool.tile([128, SPAN * F], f16, tag="junk")
                        nc.scalar.activation(
                            junk[:],
                            pa[:, g * SPAN * F:(g + 1) * SPAN * F],
                            mybir.ActivationFunctionType.Copy,
                            bias=0.0,
                            scale=1.0,
                            accum_out=racc[:, g:g + 1],
                        )
                    if t % SPAN == SPAN - 1:
                        if t == SPAN + SPAN - 1:
                            nc.vector.tensor_copy(master4[:], racc[:])
                        else:
                            nc.vector.tensor_add(master4[:], master4[:], racc[:])
                        pend_act = None
                if t == 0:
                    nc.vector.tensor_copy(master_se[:], se[:])
                else:
                    nc.vector.tensor_add(master_se[:], master_se[:], se[:])

                # PE-path mults first: the next chunk's PE reduce waits on them
                cur = []
                for g in range(PE0, G):
                    pS = prodS_pool.tile([128, F], f16, tag=f"prodS{g}")
                    nc.vector.tensor_mul(pS[:], ep[:, g * F:(g + 1) * F], w[:])
                    cur.append(pS)
                pend = (t, cur)

                # ACT-path mults (fused stt accum in the last span)
                if t < LAST_SPAN_T0:
                    if t % SPAN == 0:
                        prodA = prodA_pool.tile(
                            [128, G_ACT * SPAN * F], f16, tag="prodA"
                        )
                    for g in range(G_ACT):
                        dst = prodA[
                            :,
                            (g * SPAN + (t % SPAN)) * F:(g * SPAN + (t % SPAN) + 1) * F,
                        ]
                        nc.vector.tensor_mul(dst, ep[:, g * F:(g + 1) * F], w[:])
                    if t % SPAN == SPAN - 1:
                        racc = racc_pool.tile([128, G_ACT], fp32, tag="racc")
                        pend_act = (prodA, racc)
                else:
                    for g in range(G_ACT):
                        junk = junk_pool.tile([128, F], f16, tag="sjunk")
                        nc.vector.scalar_tensor_tensor(
                            junk[:],
                            ep[:, g * F:(g + 1) * F],
                            1.0,
                            w[:],
                            op0=mybir.AluOpType.mult,
                            op1=mybir.AluOpType.mult,
                            accum_out=stt_acc[:, (t - LAST_SPAN_T0) * G_ACT + g:
                                              (t - LAST_SPAN_T0) * G_ACT + g + 1],
                        )

            # drain: last chunk's PE reduce, stt-tail accums, final combine
            emit_pe_reduce(*pend)
            assert pend_act is None
            for sl in range(SPAN):
                nc.vector.tensor_add(
                    master4[:], master4[:],
                    stt_acc[:, sl * G_ACT:(sl + 1) * G_ACT],
                )

            res = master_pool.tile([128, RES_COLS], fp32, tag="res")
            nc.vector.tensor_copy(res[:, 0:G_ACT], master4[:])
            for gi in range(G_PE):
                nc.vector.tensor_reduce(
                    res[:, G_ACT + gi:G_ACT + gi + 1],
                    accP[gi][:],
                    axis=mybir.AxisListType.X,
                    op=mybir.AluOpType.add,
                )
            nc.vector.tensor_copy(res[:, 8:9], master_se[:])
            nc.sync.dma_start(out_dram[:], res[:])

    nc.compile()
    return nc


def _pack_core(e_core):
    """[N_PER_CORE, 64] fp32 -> [128, COLS] fp16 in the (j, e, g) layout."""
    a = e_core.reshape(N_CHUNKS, F, J, G, 8)        # [ch, fl, j, g, e]
    a = a.transpose(0, 2, 4, 3, 1)                  # [ch, j, e, g, fl]
    a = a.reshape(N_CHUNKS, 128, CH_COLS)
    a = a.transpose(1, 0, 2).reshape(128, COLS)
    return np.ascontiguousarray(a).astype(np.float16)


def _make_consts(query):
    import ml_dtypes

    c_shift = float(6.0 * np.linalg.norm(query))
    qmats = np.zeros((128, 128 * G), dtype=np.float32)
    for g in range(G):
        for j in range(J):
            qmats[8 * j:8 * j + 8, g * 128 + 8 * j:g * 128 + 8 * j + 8] = (
                query[8 * g:8 * g + 8][:, None]
            )
    ident = np.eye(128, dtype=np.float32)
    cshift = np.full((128, 1), -c_shift, dtype=np.float32)
    return (
        qmats.astype(np.float16),
        ident.astype(ml_dtypes.float8_e4m3),
        cshift,
    )


def build_in_maps(embeddings, query):
    embeddings = np.asarray(embeddings, dtype=np.float32)
    query = np.asarray(query, dtype=np.float32)
    qmats, ident, cshift = _make_consts(query)
    in_maps = []
    for c in range(N_CORES):
        e_core = embeddings[c * N_PER_CORE:(c + 1) * N_PER_CORE]
        in_maps.append({
            "epack": _pack_core(e_core),
            "qmats": qmats,
            "ident": ident,
            "cshift": cshift,
        })
    return in_maps


def combine_results(results):
    num = np.zeros(D, dtype=np.float64)
    z = 0.0
    for r in results:
        o = r["out"].astype(np.float64)              # [128, 9]
        for g in range(G):
            num[8 * g:8 * g + 8] += o[:, g].reshape(J, 8).sum(axis=0)
        z += o[0:128:8, 8].sum()
    return (num / z).astype(np.float32)


def get_nc():
    if "nc" not in _compiled:
        _compiled["nc"] = _build_nc()
    return _compiled["nc"]


def kernel(embeddings, query):
    from concourse.bass_utils import run_bass_kernel_spmd

    nc = get_nc()
    in_maps = build_in_maps(embeddings, query)

    res = None
    for attempt in range(3):
        try:
            res = run_bass_kernel_spmd(nc, in_maps, list(range(N_CORES)))
            break
        except Exception:
            if attempt == 2:
                raise

    return combine_results(res.results)
